# revision 30
# baseline (speedup 1.0000x reference)
"""Single-head causal attention on 8 TRN2 NeuronCores.

Problem: x[8,2048,1024] @ Wq/Wk/Wv[1024,64] -> causal softmax attention -> out[8,2048,64].
Sharding: data-parallel over batch B=8, one batch element per core; weights replicated.

Per-core design v8 (T=2048, C=1024, H=64):
 - x and the weights are cast to bf16 and pre-packed on the HOST (numpy), so no
   on-chip casts and no software-DGE descriptor storms.
 - HYBRID xT production: chunks 0/1 arrive early via ordinary fast DMA
   (natural layout, Act queue) and are PE-transposed during otherwise-idle PE
   time; chunks 2/3 use the DMA XBAR transpose engine (SP queue), whose
   one-time ~7us ucode warmup is triggered by a tiny dummy transpose issued
   first, and whose ~160GB/s serial delivery hides behind chunk-0/1 compute.
 - projections per chunk: pass1 stationary [Wq|Wv], pass2 [Wk|Wk]. pass2 costs
   the same as [Wk] (matmul cost = moving width) but lands kT at BOTH PSUM
   partition ranges, so the score matmuls can be ROW-PACKED: even s-blocks run
   in PE rows 0-63 (base-0 kT/qT) CONCURRENTLY with odd s-blocks in rows
   64-127 (base-64 kT, and a per-chunk SBUF->SBUF DMA partition-shifts qT up).
 - q and v leave pass1 PSUM in a single [128,CH] copy into qvT (q rows 0:64
   persist, v rows 64:128 feed tile_position=(64,0) transposes into natural
   [s,64] blocks, extended with a ones column for the softmax denominator).
 - scores are computed TRANSPOSED: weiT[s, t-chunk] = kT_si.T @ qT, diagonal
   blocks at partial width [lo:CH]; exp runs on PAIRS of s-blocks as single
   1024-wide activations; left-of-diagonal w columns may hold exp(stale-PSUM)
   garbage which PV never reads (rhs sliced [lo:CH]); the diagonal 128x128
   gets the 0/1 staircase multiply.
 - PV uses stationary [v | 1] so PSUM row 64 accumulates the softmax
   denominators; projections (and chunk-1 transposes) of chunk tb+1 are
   interleaved into the attention stream of chunk tb so the PE never idles
   while ScalarE exps.
 - warmup matmuls on memset tiles run during the first DMA to flip the PE HAM
   clock-gate to 2.4 GHz before real work lands.
"""

import numpy as np

import concourse.bass as bass
import concourse.mybir as mybir
import concourse.tile as tile
from concourse import bacc
from concourse.masks import make_identity, make_upper_triangular
from contextlib import ExitStack

P = 128
T = 2048
C = 1024
H = 64
B = 8
NC = C // P          # 8 c-tiles
NT = T // P          # 16 s/t 128-blocks
CH = 512             # t-chunk width
NCH = T // CH        # 4 chunks
BPC = CH // P        # 4 blocks per chunk
HC = CH // 2         # half chunk (pipeline-fill XBAR splits)
SCALE = float(C) ** -0.5
F32 = mybir.dt.float32
BF16 = mybir.dt.bfloat16
EXP = mybir.ActivationFunctionType.Exp


def build_nc():
    nc = bacc.Bacc(None, target_bir_lowering=False)
    x = nc.dram_tensor("x", [T, C], BF16, kind="ExternalInput")
    # host-packed stationaries: wqv[c%128, c//128, 0:64]=Wq, [.., 64:128]=Wv
    wqv_d = nc.dram_tensor("wqv", [P, NC, P], BF16, kind="ExternalInput")
    wkk_d = nc.dram_tensor("wkk", [P, NC, P], BF16, kind="ExternalInput")
    out_d = nc.dram_tensor("outT", [H + 1, T], F32, kind="ExternalOutput")

    with tile.TileContext(nc) as tc, ExitStack() as ctx:
        consts = ctx.enter_context(tc.tile_pool(name="consts", bufs=1))
        persist = ctx.enter_context(tc.tile_pool(name="persist", bufs=1))
        wei = ctx.enter_context(tc.tile_pool(name="wei", bufs=4))
        fin = ctx.enter_context(tc.tile_pool(name="fin", bufs=2))
        # PSUM: 8 banks, per (pool, tag): ppj/p1 1 + ppj/p2 1 + ppj/vn 1 +
        # psc/sc 2x2 + pout/po 1 = 8. PE-transpose batches borrow sc slots.
        ppj = ctx.enter_context(tc.tile_pool(name="ppj", bufs=1, space="PSUM"))
        psc = ctx.enter_context(tc.tile_pool(name="psc", bufs=2, space="PSUM"))
        pout = ctx.enter_context(tc.tile_pool(name="pout", bufs=1, space="PSUM"))

        # ---- DMAs first (no deps, start immediately).
        # SP queue: dummy XBAR (eats the one-time ucode warmup), then chunks
        # 2/3 XBAR transposes. Act queue: weights, then chunk 0/1 naturals.
        # XBAR transposes serialize on a shared ucode resource with a ~7us
        # one-time warmup: a tiny dummy transpose goes first so the warmup
        # overlaps the framework preamble; then chunks in order, 0/1 split in
        # halves so their projections can start per-piece.
        xTs = []
        xpieces = []  # per chunk: list of (col_offset, width)
        for tb in range(NCH):
            xTs.append(persist.tile([P, NC, CH], BF16, tag=f"xT{tb}",
                                    name=f"xT{tb}"))
            xpieces.append([(0, CH)])
        wqv_sb = consts.tile([P, NC, P], BF16)
        nc.scalar.dma_start(out=wqv_sb, in_=wqv_d[:, :, :])
        wkk_sb = consts.tile([P, NC, P], BF16)
        nc.scalar.dma_start(out=wkk_sb, in_=wkk_d[:, :, :])
        # dummy XBAR triggers the transpose-ucode warmup during the preamble;
        # the chunk XBARs then flow gapless and land just-in-time for the
        # weave (Act-queue XBARs can't start before ~20us, so all ride SP).
        xbar_scratch = consts.tile([P, 16], BF16)
        nc.sync.dma_start(out=xbar_scratch, in_=x[0:16, 0:P], transpose=True)
        for tb in range(NCH):
            t0 = tb * CH
            nc.sync.dma_start(out=xTs[tb], in_=x[t0 : t0 + CH, :], transpose=True)

        # ---- constants
        ident_f = consts.tile([P, P], F32)
        make_identity(nc, ident_f)
        tri_f = consts.tile([P, P], F32)  # tri[s, u] = 1 if u >= s else 0
        make_upper_triangular(nc, tri_f, val=1.0, diag=True)
        ident_b = consts.tile([P, P], BF16)
        nc.vector.tensor_copy(out=ident_b, in_=ident_f)
        tri = consts.tile([P, P], BF16)
        nc.vector.tensor_copy(out=tri, in_=tri_f)

        # qvT: rows 0:64 = qT (persist), rows 64:128 = v transposed staging
        qvT = persist.tile([P, T], BF16, tag="qvT")
        qT = qvT[0:H, :]
        # kkT: rows 0:64 = kT for even s-blocks, rows 64:128 = same kT (for the
        # row-packed odd s-block matmuls at tile_position (64,0))
        kkT = persist.tile([P, T], BF16, tag="kkT")
        # qhT rows 64:128 = qT partition-shifted up via SBUF->SBUF DMA
        qhT = persist.tile([P, T], BF16, tag="qhT")
        v_all = persist.tile([P, NT, H + 1], BF16, tag="v")
        nc.vector.memset(v_all[:, :, H : H + 1], 1.0)  # softmax-denominator column

        # ---- HAM warmup: dummy matmuls on memset tiles (ready instantly) keep
        # the PE busy until chunk 0 lands so real matmuls run at 2.4 GHz.
        warm_lhs = consts.tile([P, P], BF16)
        nc.vector.memset(warm_lhs, 0.0)
        warm_in = consts.tile([P, CH], BF16)
        nc.vector.memset(warm_in, 0.0)
        pwarm = psc.tile([P, 2, CH], F32, tag="sc")
        for i in range(26):
            nc.tensor.matmul(pwarm[:, 0, :], lhsT=warm_lhs, rhs=warm_in,
                             start=True, stop=True)

        def proj_steps(tb):
            """Generator of projection work items for chunk tb (PE + DVE).
            Piece-wise over the chunk's XBAR arrivals, as ONE accumulation
            group per bank: start only on the very first matmul (clears the
            bank's has_written bits); later pieces' first writes hit
            bit-unset elements and overwrite, then accumulate."""
            tsl = slice(tb * CH, (tb + 1) * CH)
            pq = ppj.tile([P, CH], F32, tag="p1")
            pk = ppj.tile([P, CH], F32, tag="p2")
            xt = xTs[tb]
            pieces = xpieces[tb]
            np_ = len(pieces)
            for pi, (off, wd) in enumerate(pieces):
                for jc in range(NC):
                    first = pi == 0 and jc == 0
                    last = pi == np_ - 1 and jc == NC - 1
                    yield lambda off=off, wd=wd, jc=jc, first=first, last=last: (
                        nc.tensor.matmul(pq[:, off : off + wd],
                                         lhsT=wqv_sb[:, jc, :],
                                         rhs=xt[:, jc, off : off + wd],
                                         start=first, stop=last,
                                         skip_group_check=True),
                        nc.tensor.matmul(pk[:, off : off + wd],
                                         lhsT=wkk_sb[:, jc, :],
                                         rhs=xt[:, jc, off : off + wd],
                                         start=first, stop=last,
                                         skip_group_check=True),
                    )

            def tail():
                # q (rows 0:64) and v (rows 64:128) leave PSUM in one copy;
                # kT lands at both partition ranges in one copy.
                nc.vector.tensor_copy(out=qvT[:, tsl], in_=pq)
                nc.vector.tensor_copy(out=kkT[:, tsl], in_=pk)
                # partition-shift qT up for the odd (row-packed) score matmuls
                nc.scalar.dma_start(out=qhT[H:P, tsl], in_=qvT[0:H, tsl])
                pvn = ppj.tile([P, BPC, H], BF16, tag="vn")
                for tt in range(BPC):
                    c0 = tb * CH + tt * P
                    nc.tensor.transpose(pvn[:, tt, :], qvT[H:P, c0 : c0 + P],
                                        ident_b[H:P, H:P])
                nc.vector.tensor_copy(
                    out=v_all[:, tb * BPC : (tb + 1) * BPC, 0:H], in_=pvn)

            yield tail

        def attention(tb, pending):
            """Attention for chunk tb, weaving pending proj steps of tb+1 into
            the PE stream. exp runs on si pairs; even/odd score matmuls are
            row-packed into concurrent PE halves."""
            tsl = slice(tb * CH, (tb + 1) * CH)
            po = pout.tile([H + 1, CH], F32, tag="po")
            nsb = (tb + 1) * BPC
            wpairs = {}

            def weave():
                try:
                    next(pending)()
                except StopIteration:
                    pass

            def lo_of(si):
                return max(0, (si - tb * BPC) * P)

            for pr in range(nsb // 2 + 1):
                if pr < nsb // 2:
                    ps = psc.tile([P, 2, CH], F32, tag="sc")
                    w = wei.tile([P, 2, CH], BF16, tag="w")
                    se, so = 2 * pr, 2 * pr + 1
                    loe, loo = lo_of(se), lo_of(so)
                    # even s-block: PE rows 0-63; odd: rows 64-127 (concurrent)
                    nc.tensor.matmul(ps[:, 0, loe:CH],
                                     lhsT=kkT[0:H, se * P : (se + 1) * P],
                                     rhs=qT[:, tb * CH + loe : (tb + 1) * CH],
                                     start=True, stop=True)
                    nc.tensor.matmul(ps[:, 1, loo:CH],
                                     lhsT=kkT[H:P, so * P : (so + 1) * P],
                                     rhs=qhT[H:P, tb * CH + loo : (tb + 1) * CH],
                                     start=True, stop=True)
                    weave()
                    nc.scalar.activation(out=w, in_=ps, func=EXP, scale=SCALE)
                    for half in range(2):
                        si = 2 * pr + half
                        if si >= tb * BPC:  # diagonal 128x128: staircase mask
                            lo = lo_of(si)
                            nc.vector.tensor_mul(w[:, half, lo : lo + P],
                                                 w[:, half, lo : lo + P], tri)
                    wpairs[pr] = w
                if pr > 0:
                    w = wpairs.pop(pr - 1)
                    for half in range(2):
                        si = 2 * (pr - 1) + half
                        lo = lo_of(si)
                        nc.tensor.matmul(po[:, lo:CH], lhsT=v_all[:, si, :],
                                         rhs=w[:, half, lo:CH],
                                         start=(si == 0), stop=(si == nsb - 1))
                    weave()
            # finalize chunk: copy outT+sums to SBUF and store; the per-row
            # divide + transpose happens host-side during unshard.
            os_ = fin.tile([H + 1, CH], F32, tag="ot")
            nc.vector.tensor_copy(out=os_, in_=po)
            nc.gpsimd.dma_start(out=out_d[:, tsl], in_=os_)

        # chunk 0 projections run un-weaved; chunk tb+1 projections weave into
        # chunk tb's attention stream.
        for step in proj_steps(0):
            step()
        for tb in range(NCH):
            pending = proj_steps(tb + 1) if tb + 1 < NCH else iter(())
            attention(tb, pending)
            for step in pending:  # any proj work not yet woven
                step()
    return nc


_NC_CACHE = []


def _get_nc():
    if not _NC_CACHE:
        nc = build_nc()
        nc.finalize()  # bacc compile: register allocation, DCE
        _NC_CACHE.append(nc)
    return _NC_CACHE[0]


def _pack_inputs(x, wq, wk, wv):
    import ml_dtypes

    bf16 = np.dtype(ml_dtypes.bfloat16)
    # stationary packing: [c%128, c//128, h]; pass1 = [Wq | Wv], pass2 = [Wk | Wk]
    wq_p = wq.reshape(NC, P, H).transpose(1, 0, 2)
    wv_p = wv.reshape(NC, P, H).transpose(1, 0, 2)
    wk_p = wk.reshape(NC, P, H).transpose(1, 0, 2)
    wqv = np.ascontiguousarray(np.concatenate([wq_p, wv_p], axis=2)).astype(bf16)
    wkk = np.ascontiguousarray(np.concatenate([wk_p, wk_p], axis=2)).astype(bf16)
    xb = np.ascontiguousarray(x).astype(bf16)
    return xb, wqv, wkk


def kernel(**inputs):
    x = np.asarray(inputs["x"], dtype=np.float32)
    wq = np.asarray(inputs["Wq"], dtype=np.float32)
    wk = np.asarray(inputs["Wk"], dtype=np.float32)
    wv = np.asarray(inputs["Wv"], dtype=np.float32)
    from concourse.bass_utils import run_bass_kernel_spmd

    nc = _get_nc()
    xb, wqv, wkk = _pack_inputs(x, wq, wk, wv)
    in_maps = [{"x": np.ascontiguousarray(xb[b]), "wqv": wqv, "wkk": wkk}
               for b in range(B)]
    res = run_bass_kernel_spmd(nc, in_maps, core_ids=list(range(B)))
    return postprocess([res.results[b]["outT"] for b in range(B)])


def postprocess(outTs):
    outs = []
    for oT in outTs:
        outs.append((oT[0:H, :] / oT[H : H + 1, :]).T.astype(np.float32))
    return np.stack(outs, axis=0)


if __name__ == "__main__":
    import os
    os.makedirs("/tmp/neffdir3", exist_ok=True)
    from concourse.bass_utils import compile_bass_kernel

    nc = _get_nc()
    print("build OK, instructions:",
          sum(len(bb.instructions) for bb in nc.m.functions[0].blocks))
    print("COMPILED:", compile_bass_kernel(nc, "/tmp/neffdir3"))


# revision 31
# speedup vs baseline: 1.2665x; 1.2665x over previous
"""Single-head causal attention on 8 TRN2 NeuronCores.

Problem: x[8,2048,1024] @ Wq/Wk/Wv[1024,64] -> causal softmax attention -> out[8,2048,64].
Sharding: data-parallel over batch B=8, one batch element per core; weights replicated.

Per-core design v8 (T=2048, C=1024, H=64):
 - x and the weights are cast to bf16 and pre-packed on the HOST (numpy), so no
   on-chip casts and no software-DGE descriptor storms.
 - HYBRID xT production: chunks 0/1 arrive early via ordinary fast DMA
   (natural layout, Act queue) and are PE-transposed during otherwise-idle PE
   time; chunks 2/3 use the DMA XBAR transpose engine (SP queue), whose
   one-time ~7us ucode warmup is triggered by a tiny dummy transpose issued
   first, and whose ~160GB/s serial delivery hides behind chunk-0/1 compute.
 - projections per chunk: pass1 stationary [Wq|Wv], pass2 [Wk|Wk]. pass2 costs
   the same as [Wk] (matmul cost = moving width) but lands kT at BOTH PSUM
   partition ranges, so the score matmuls can be ROW-PACKED: even s-blocks run
   in PE rows 0-63 (base-0 kT/qT) CONCURRENTLY with odd s-blocks in rows
   64-127 (base-64 kT, and a per-chunk SBUF->SBUF DMA partition-shifts qT up).
 - q and v leave pass1 PSUM in a single [128,CH] copy into qvT (q rows 0:64
   persist, v rows 64:128 feed tile_position=(64,0) transposes into natural
   [s,64] blocks, extended with a ones column for the softmax denominator).
 - scores are computed TRANSPOSED: weiT[s, t-chunk] = kT_si.T @ qT, diagonal
   blocks at partial width [lo:CH]; exp runs on PAIRS of s-blocks as single
   1024-wide activations; left-of-diagonal w columns may hold exp(stale-PSUM)
   garbage which PV never reads (rhs sliced [lo:CH]); the diagonal 128x128
   gets the 0/1 staircase multiply.
 - PV uses stationary [v | 1] so PSUM row 64 accumulates the softmax
   denominators; projections (and chunk-1 transposes) of chunk tb+1 are
   interleaved into the attention stream of chunk tb so the PE never idles
   while ScalarE exps.
 - warmup matmuls on memset tiles run during the first DMA to flip the PE HAM
   clock-gate to 2.4 GHz before real work lands.
"""

import numpy as np

import concourse.bass as bass
import concourse.mybir as mybir
import concourse.tile as tile
from concourse import bacc
from concourse.masks import make_identity, make_upper_triangular
from contextlib import ExitStack

P = 128
T = 2048
C = 1024
H = 64
B = 8
NC = C // P          # 8 c-tiles
NT = T // P          # 16 s/t 128-blocks
CH = 512             # t-chunk width
NCH = T // CH        # 4 chunks
BPC = CH // P        # 4 blocks per chunk
HC = CH // 2         # half chunk (pipeline-fill XBAR splits)
SCALE = float(C) ** -0.5
F32 = mybir.dt.float32
BF16 = mybir.dt.bfloat16
EXP = mybir.ActivationFunctionType.Exp


def build_nc():
    nc = bacc.Bacc(None, target_bir_lowering=False)
    # host-transposed x tiles: xt[tb, c%128, c//128, t_in_chunk]
    xt_d = nc.dram_tensor("xt", [NCH, P, NC, CH], BF16, kind="ExternalInput")
    # host-packed stationaries: wqv[c%128, c//128, 0:64]=Wq, [.., 64:128]=Wv
    wqv_d = nc.dram_tensor("wqv", [P, NC, P], BF16, kind="ExternalInput")
    wkk_d = nc.dram_tensor("wkk", [P, NC, P], BF16, kind="ExternalInput")
    out_d = nc.dram_tensor("outT", [H + 1, T], F32, kind="ExternalOutput")

    with tile.TileContext(nc) as tc, ExitStack() as ctx:
        consts = ctx.enter_context(tc.tile_pool(name="consts", bufs=1))
        persist = ctx.enter_context(tc.tile_pool(name="persist", bufs=1))
        wei = ctx.enter_context(tc.tile_pool(name="wei", bufs=4))
        fin = ctx.enter_context(tc.tile_pool(name="fin", bufs=2))
        # PSUM: 8 banks, per (pool, tag): ppj/p1 1 + ppj/p2 1 + ppj/vn 1 +
        # psc/sc 2x2 + pout/po 1 = 8. PE-transpose batches borrow sc slots.
        ppj = ctx.enter_context(tc.tile_pool(name="ppj", bufs=1, space="PSUM"))
        psc = ctx.enter_context(tc.tile_pool(name="psc", bufs=2, space="PSUM"))
        pout = ctx.enter_context(tc.tile_pool(name="pout", bufs=1, space="PSUM"))

        # ---- DMAs first (no deps, start immediately).
        # SP queue: dummy XBAR (eats the one-time ucode warmup), then chunks
        # 2/3 XBAR transposes. Act queue: weights, then chunk 0/1 naturals.
        # XBAR transposes serialize on a shared ucode resource with a ~7us
        # one-time warmup: a tiny dummy transpose goes first so the warmup
        # overlaps the framework preamble; then chunks in order, 0/1 split in
        # halves so their projections can start per-piece.
        xTs = []
        xpieces = []  # per chunk: list of (col_offset, width)
        for tb in range(NCH):
            xTs.append(persist.tile([P, NC, CH], BF16, tag=f"xT{tb}",
                                    name=f"xT{tb}"))
            xpieces.append([(0, CH)])
        wqv_sb = consts.tile([P, NC, P], BF16)
        nc.scalar.dma_start(out=wqv_sb, in_=wqv_d[:, :, :])
        wkk_sb = consts.tile([P, NC, P], BF16)
        nc.scalar.dma_start(out=wkk_sb, in_=wkk_d[:, :, :])
        # xT chunks are host-transposed and land via plain full-speed DMAs
        # (8KB contiguous per partition), alternating the two HWDGE queues.
        for tb in range(NCH):
            eng = nc.sync if tb % 2 == 0 else nc.scalar
            eng.dma_start(out=xTs[tb], in_=xt_d[tb, :, :, :])

        # ---- constants
        ident_f = consts.tile([P, P], F32)
        make_identity(nc, ident_f)
        tri_f = consts.tile([P, P], F32)  # tri[s, u] = 1 if u >= s else 0
        make_upper_triangular(nc, tri_f, val=1.0, diag=True)
        ident_b = consts.tile([P, P], BF16)
        nc.vector.tensor_copy(out=ident_b, in_=ident_f)
        tri = consts.tile([P, P], BF16)
        nc.vector.tensor_copy(out=tri, in_=tri_f)

        # qvT: rows 0:64 = qT (persist), rows 64:128 = v transposed staging
        qvT = persist.tile([P, T], BF16, tag="qvT")
        qT = qvT[0:H, :]
        # kkT: rows 0:64 = kT for even s-blocks, rows 64:128 = same kT (for the
        # row-packed odd s-block matmuls at tile_position (64,0))
        kkT = persist.tile([P, T], BF16, tag="kkT")
        # qhT rows 64:128 = qT partition-shifted up via SBUF->SBUF DMA
        qhT = persist.tile([P, T], BF16, tag="qhT")
        v_all = persist.tile([P, NT, H + 1], BF16, tag="v")
        nc.vector.memset(v_all[:, :, H : H + 1], 1.0)  # softmax-denominator column

        # ---- HAM warmup: dummy matmuls on memset tiles (ready instantly) keep
        # the PE busy until chunk 0 lands so real matmuls run at 2.4 GHz.
        warm_lhs = consts.tile([P, P], BF16)
        nc.vector.memset(warm_lhs, 0.0)
        warm_in = consts.tile([P, CH], BF16)
        nc.vector.memset(warm_in, 0.0)
        pwarm = psc.tile([P, 2, CH], F32, tag="sc")
        for i in range(12):
            nc.tensor.matmul(pwarm[:, 0, :], lhsT=warm_lhs, rhs=warm_in,
                             start=True, stop=True)

        def proj_steps(tb):
            """Generator of projection work items for chunk tb (PE + DVE).
            Piece-wise over the chunk's XBAR arrivals, as ONE accumulation
            group per bank: start only on the very first matmul (clears the
            bank's has_written bits); later pieces' first writes hit
            bit-unset elements and overwrite, then accumulate."""
            tsl = slice(tb * CH, (tb + 1) * CH)
            pq = ppj.tile([P, CH], F32, tag="p1")
            pk = ppj.tile([P, CH], F32, tag="p2")
            xt = xTs[tb]
            pieces = xpieces[tb]
            np_ = len(pieces)
            for pi, (off, wd) in enumerate(pieces):
                for jc in range(NC):
                    first = pi == 0 and jc == 0
                    last = pi == np_ - 1 and jc == NC - 1
                    yield lambda off=off, wd=wd, jc=jc, first=first, last=last: (
                        nc.tensor.matmul(pq[:, off : off + wd],
                                         lhsT=wqv_sb[:, jc, :],
                                         rhs=xt[:, jc, off : off + wd],
                                         start=first, stop=last,
                                         skip_group_check=True),
                        nc.tensor.matmul(pk[:, off : off + wd],
                                         lhsT=wkk_sb[:, jc, :],
                                         rhs=xt[:, jc, off : off + wd],
                                         start=first, stop=last,
                                         skip_group_check=True),
                    )

            def tail():
                # q (rows 0:64) and v (rows 64:128) leave PSUM in one copy;
                # kT lands at both partition ranges in one copy.
                nc.vector.tensor_copy(out=qvT[:, tsl], in_=pq)
                nc.vector.tensor_copy(out=kkT[:, tsl], in_=pk)
                # partition-shift qT up for the odd (row-packed) score matmuls
                nc.scalar.dma_start(out=qhT[H:P, tsl], in_=qvT[0:H, tsl])
                pvn = ppj.tile([P, BPC, H], BF16, tag="vn")
                for tt in range(BPC):
                    c0 = tb * CH + tt * P
                    nc.tensor.transpose(pvn[:, tt, :], qvT[H:P, c0 : c0 + P],
                                        ident_b[H:P, H:P])
                nc.vector.tensor_copy(
                    out=v_all[:, tb * BPC : (tb + 1) * BPC, 0:H], in_=pvn)

            yield tail

        def attention(tb, pending):
            """Attention for chunk tb, weaving pending proj steps of tb+1 into
            the PE stream. exp runs on si pairs; even/odd score matmuls are
            row-packed into concurrent PE halves."""
            tsl = slice(tb * CH, (tb + 1) * CH)
            po = pout.tile([H + 1, CH], F32, tag="po")
            nsb = (tb + 1) * BPC
            wpairs = {}

            def weave():
                try:
                    next(pending)()
                except StopIteration:
                    pass

            def lo_of(si):
                return max(0, (si - tb * BPC) * P)

            for pr in range(nsb // 2 + 1):
                if pr < nsb // 2:
                    ps = psc.tile([P, 2, CH], F32, tag="sc")
                    w = wei.tile([P, 2, CH], BF16, tag="w")
                    se, so = 2 * pr, 2 * pr + 1
                    loe, loo = lo_of(se), lo_of(so)
                    # even s-block: PE rows 0-63; odd: rows 64-127 (concurrent)
                    nc.tensor.matmul(ps[:, 0, loe:CH],
                                     lhsT=kkT[0:H, se * P : (se + 1) * P],
                                     rhs=qT[:, tb * CH + loe : (tb + 1) * CH],
                                     start=True, stop=True)
                    nc.tensor.matmul(ps[:, 1, loo:CH],
                                     lhsT=kkT[H:P, so * P : (so + 1) * P],
                                     rhs=qhT[H:P, tb * CH + loo : (tb + 1) * CH],
                                     start=True, stop=True)
                    weave()
                    nc.scalar.activation(out=w, in_=ps, func=EXP, scale=SCALE)
                    for half in range(2):
                        si = 2 * pr + half
                        if si >= tb * BPC:  # diagonal 128x128: staircase mask
                            lo = lo_of(si)
                            nc.vector.tensor_mul(w[:, half, lo : lo + P],
                                                 w[:, half, lo : lo + P], tri)
                    wpairs[pr] = w
                if pr > 0:
                    w = wpairs.pop(pr - 1)
                    for half in range(2):
                        si = 2 * (pr - 1) + half
                        lo = lo_of(si)
                        nc.tensor.matmul(po[:, lo:CH], lhsT=v_all[:, si, :],
                                         rhs=w[:, half, lo:CH],
                                         start=(si == 0), stop=(si == nsb - 1))
                    weave()
            # finalize chunk: copy outT+sums to SBUF and store; the per-row
            # divide + transpose happens host-side during unshard.
            os_ = fin.tile([H + 1, CH], F32, tag="ot")
            nc.vector.tensor_copy(out=os_, in_=po)
            nc.gpsimd.dma_start(out=out_d[:, tsl], in_=os_)

        # chunk 0 projections run un-weaved; chunk tb+1 projections weave into
        # chunk tb's attention stream.
        for step in proj_steps(0):
            step()
        for tb in range(NCH):
            pending = proj_steps(tb + 1) if tb + 1 < NCH else iter(())
            attention(tb, pending)
            for step in pending:  # any proj work not yet woven
                step()
    return nc


_NC_CACHE = []


def _get_nc():
    if not _NC_CACHE:
        nc = build_nc()
        nc.finalize()  # bacc compile: register allocation, DCE
        _NC_CACHE.append(nc)
    return _NC_CACHE[0]


def _pack_inputs(x, wq, wk, wv):
    import ml_dtypes

    bf16 = np.dtype(ml_dtypes.bfloat16)
    # stationary packing: [c%128, c//128, h]; pass1 = [Wq | Wv], pass2 = [Wk | Wk]
    wq_p = wq.reshape(NC, P, H).transpose(1, 0, 2)
    wv_p = wv.reshape(NC, P, H).transpose(1, 0, 2)
    wk_p = wk.reshape(NC, P, H).transpose(1, 0, 2)
    wqv = np.ascontiguousarray(np.concatenate([wq_p, wv_p], axis=2)).astype(bf16)
    wkk = np.ascontiguousarray(np.concatenate([wk_p, wk_p], axis=2)).astype(bf16)
    # xt[b, tb, c%128, c//128, t_in_chunk] = x[b, tb*CH + t, c]
    xb = x.astype(bf16)                      # [B, T, C]
    xt = xb.transpose(0, 2, 1)               # [B, C, T]
    xt = xt.reshape(B, NC, P, NCH, CH)       # [B, jc, p, tb, t]
    xt = np.ascontiguousarray(xt.transpose(0, 3, 2, 1, 4))  # [B, tb, p, jc, t]
    return xt, wqv, wkk


def kernel(**inputs):
    x = np.asarray(inputs["x"], dtype=np.float32)
    wq = np.asarray(inputs["Wq"], dtype=np.float32)
    wk = np.asarray(inputs["Wk"], dtype=np.float32)
    wv = np.asarray(inputs["Wv"], dtype=np.float32)
    from concourse.bass_utils import run_bass_kernel_spmd

    nc = _get_nc()
    xb, wqv, wkk = _pack_inputs(x, wq, wk, wv)
    in_maps = [{"xt": np.ascontiguousarray(xb[b]), "wqv": wqv, "wkk": wkk}
               for b in range(B)]
    res = run_bass_kernel_spmd(nc, in_maps, core_ids=list(range(B)))
    return postprocess([res.results[b]["outT"] for b in range(B)])


def postprocess(outTs):
    outs = []
    for oT in outTs:
        outs.append((oT[0:H, :] / oT[H : H + 1, :]).T.astype(np.float32))
    return np.stack(outs, axis=0)


if __name__ == "__main__":
    import os
    os.makedirs("/tmp/neffdir3", exist_ok=True)
    from concourse.bass_utils import compile_bass_kernel

    nc = _get_nc()
    print("build OK, instructions:",
          sum(len(bb.instructions) for bb in nc.m.functions[0].blocks))
    print("COMPILED:", compile_bass_kernel(nc, "/tmp/neffdir3"))


# revision 32
# speedup vs baseline: 1.2813x; 1.0116x over previous
"""Single-head causal attention on 8 TRN2 NeuronCores.

Problem: x[8,2048,1024] @ Wq/Wk/Wv[1024,64] -> causal softmax attention -> out[8,2048,64].
Sharding: data-parallel over batch B=8, one batch element per core; weights replicated.

Per-core design v8 (T=2048, C=1024, H=64):
 - x and the weights are cast to bf16 and pre-packed on the HOST (numpy), so no
   on-chip casts and no software-DGE descriptor storms.
 - HYBRID xT production: chunks 0/1 arrive early via ordinary fast DMA
   (natural layout, Act queue) and are PE-transposed during otherwise-idle PE
   time; chunks 2/3 use the DMA XBAR transpose engine (SP queue), whose
   one-time ~7us ucode warmup is triggered by a tiny dummy transpose issued
   first, and whose ~160GB/s serial delivery hides behind chunk-0/1 compute.
 - projections per chunk: pass1 stationary [Wq|Wv], pass2 [Wk|Wk]. pass2 costs
   the same as [Wk] (matmul cost = moving width) but lands kT at BOTH PSUM
   partition ranges, so the score matmuls can be ROW-PACKED: even s-blocks run
   in PE rows 0-63 (base-0 kT/qT) CONCURRENTLY with odd s-blocks in rows
   64-127 (base-64 kT, and a per-chunk SBUF->SBUF DMA partition-shifts qT up).
 - q and v leave pass1 PSUM in a single [128,CH] copy into qvT (q rows 0:64
   persist, v rows 64:128 feed tile_position=(64,0) transposes into natural
   [s,64] blocks, extended with a ones column for the softmax denominator).
 - scores are computed TRANSPOSED: weiT[s, t-chunk] = kT_si.T @ qT, diagonal
   blocks at partial width [lo:CH]; exp runs on PAIRS of s-blocks as single
   1024-wide activations; left-of-diagonal w columns may hold exp(stale-PSUM)
   garbage which PV never reads (rhs sliced [lo:CH]); the diagonal 128x128
   gets the 0/1 staircase multiply.
 - PV uses stationary [v | 1] so PSUM row 64 accumulates the softmax
   denominators; projections (and chunk-1 transposes) of chunk tb+1 are
   interleaved into the attention stream of chunk tb so the PE never idles
   while ScalarE exps.
 - warmup matmuls on memset tiles run during the first DMA to flip the PE HAM
   clock-gate to 2.4 GHz before real work lands.
"""

import numpy as np

import concourse.bass as bass
import concourse.mybir as mybir
import concourse.tile as tile
from concourse import bacc
from concourse.masks import make_identity, make_upper_triangular
from contextlib import ExitStack

P = 128
T = 2048
C = 1024
H = 64
B = 8
NC = C // P          # 8 c-tiles
NT = T // P          # 16 s/t 128-blocks
CH = 512             # t-chunk width
NCH = T // CH        # 4 chunks
BPC = CH // P        # 4 blocks per chunk
HC = CH // 2         # half chunk (pipeline-fill XBAR splits)
SCALE = float(C) ** -0.5
F32 = mybir.dt.float32
BF16 = mybir.dt.bfloat16
EXP = mybir.ActivationFunctionType.Exp


def build_nc():
    nc = bacc.Bacc(None, target_bir_lowering=False)
    # host-transposed x tiles: xt[tb, c%128, c//128, t_in_chunk]
    xt_d = nc.dram_tensor("xt", [NCH, P, NC, CH], BF16, kind="ExternalInput")
    # host-packed stationaries: wqv[c%128, c//128, 0:64]=Wq, [.., 64:128]=Wv
    wqv_d = nc.dram_tensor("wqv", [P, NC, P], BF16, kind="ExternalInput")
    wkk_d = nc.dram_tensor("wkk", [P, NC, P], BF16, kind="ExternalInput")
    out_d = nc.dram_tensor("outT", [H + 1, T], F32, kind="ExternalOutput")

    with tile.TileContext(nc) as tc, ExitStack() as ctx:
        consts = ctx.enter_context(tc.tile_pool(name="consts", bufs=1))
        persist = ctx.enter_context(tc.tile_pool(name="persist", bufs=1))
        wei = ctx.enter_context(tc.tile_pool(name="wei", bufs=4))
        fin = ctx.enter_context(tc.tile_pool(name="fin", bufs=2))
        # PSUM: 8 banks, per (pool, tag): ppj/p1 1 + ppj/p2 1 +
        # psc/sc 2x2 + pout/po 2 = 8. The v mini-transposes borrow sc slots.
        ppj = ctx.enter_context(tc.tile_pool(name="ppj", bufs=1, space="PSUM"))
        psc = ctx.enter_context(tc.tile_pool(name="psc", bufs=2, space="PSUM"))
        pout = ctx.enter_context(tc.tile_pool(name="pout", bufs=2, space="PSUM"))

        # ---- DMAs first (no deps, start immediately).
        # SP queue: dummy XBAR (eats the one-time ucode warmup), then chunks
        # 2/3 XBAR transposes. Act queue: weights, then chunk 0/1 naturals.
        # XBAR transposes serialize on a shared ucode resource with a ~7us
        # one-time warmup: a tiny dummy transpose goes first so the warmup
        # overlaps the framework preamble; then chunks in order, 0/1 split in
        # halves so their projections can start per-piece.
        xTs = []
        xpieces = []  # per chunk: list of (col_offset, width)
        for tb in range(NCH):
            xTs.append(persist.tile([P, NC, CH], BF16, tag=f"xT{tb}",
                                    name=f"xT{tb}"))
            xpieces.append([(0, CH)])
        wqv_sb = consts.tile([P, NC, P], BF16)
        nc.scalar.dma_start(out=wqv_sb, in_=wqv_d[:, :, :])
        wkk_sb = consts.tile([P, NC, P], BF16)
        nc.scalar.dma_start(out=wkk_sb, in_=wkk_d[:, :, :])
        # xT chunks are host-transposed and land via plain full-speed DMAs
        # (8KB contiguous per partition), alternating the two HWDGE queues.
        nc.sync.dma_start(out=xTs[0][:, 0:4, :], in_=xt_d[0, :, 0:4, :])
        nc.scalar.dma_start(out=xTs[0][:, 4:8, :], in_=xt_d[0, :, 4:8, :])
        for tb in range(1, NCH):
            eng = nc.sync if tb % 2 == 0 else nc.scalar
            eng.dma_start(out=xTs[tb], in_=xt_d[tb, :, :, :])

        # ---- constants
        ident_f = consts.tile([P, P], F32)
        make_identity(nc, ident_f)
        tri_f = consts.tile([P, P], F32)  # tri[s, u] = 1 if u >= s else 0
        make_upper_triangular(nc, tri_f, val=1.0, diag=True)
        ident_b = consts.tile([P, P], BF16)
        nc.vector.tensor_copy(out=ident_b, in_=ident_f)
        tri = consts.tile([P, P], BF16)
        nc.vector.tensor_copy(out=tri, in_=tri_f)

        # qvT: rows 0:64 = qT (persist), rows 64:128 = v transposed staging
        qvT = persist.tile([P, T], BF16, tag="qvT")
        qT = qvT[0:H, :]
        # kkT: rows 0:64 = kT for even s-blocks, rows 64:128 = same kT (for the
        # row-packed odd s-block matmuls at tile_position (64,0))
        kkT = persist.tile([P, T], BF16, tag="kkT")
        # qhT rows 64:128 = qT partition-shifted up via SBUF->SBUF DMA
        qhT = persist.tile([P, T], BF16, tag="qhT")
        v_all = persist.tile([P, NT, H + 1], BF16, tag="v")
        nc.vector.memset(v_all[:, :, H : H + 1], 1.0)  # softmax-denominator column

        # ---- HAM warmup: dummy matmuls on memset tiles (ready instantly) keep
        # the PE busy until chunk 0 lands so real matmuls run at 2.4 GHz.
        warm_lhs = consts.tile([P, P], BF16)
        nc.vector.memset(warm_lhs, 0.0)
        warm_in = consts.tile([P, CH], BF16)
        nc.vector.memset(warm_in, 0.0)
        # preload the ScalarE exp table (~2.7us) off the critical path
        warm_act = consts.tile([P, 1], BF16)
        nc.scalar.activation(out=warm_act, in_=warm_in[:, 0:1], func=EXP,
                             scale=1.0)
        pwarm = psc.tile([P, 2, CH], F32, tag="sc")
        for i in range(16):
            nc.tensor.matmul(pwarm[:, 0, :], lhsT=warm_lhs, rhs=warm_in,
                             start=True, stop=True)

        def proj_steps(tb):
            """Generator of projection work items for chunk tb (PE + DVE).
            Piece-wise over the chunk's XBAR arrivals, as ONE accumulation
            group per bank: start only on the very first matmul (clears the
            bank's has_written bits); later pieces' first writes hit
            bit-unset elements and overwrite, then accumulate."""
            tsl = slice(tb * CH, (tb + 1) * CH)
            pq = ppj.tile([P, CH], F32, tag="p1")
            pk = ppj.tile([P, CH], F32, tag="p2")
            xt = xTs[tb]
            pieces = xpieces[tb]
            np_ = len(pieces)
            for pi, (off, wd) in enumerate(pieces):
                for jc in range(NC):
                    first = pi == 0 and jc == 0
                    last = pi == np_ - 1 and jc == NC - 1
                    yield lambda off=off, wd=wd, jc=jc, first=first, last=last: (
                        nc.tensor.matmul(pq[:, off : off + wd],
                                         lhsT=wqv_sb[:, jc, :],
                                         rhs=xt[:, jc, off : off + wd],
                                         start=first, stop=last,
                                         skip_group_check=True),
                        nc.tensor.matmul(pk[:, off : off + wd],
                                         lhsT=wkk_sb[:, jc, :],
                                         rhs=xt[:, jc, off : off + wd],
                                         start=first, stop=last,
                                         skip_group_check=True),
                    )

            def tail():
                # q (rows 0:64) and v (rows 64:128) leave PSUM in one copy;
                # kT lands at both partition ranges in one copy.
                nc.vector.tensor_copy(out=qvT[:, tsl], in_=pq)
                nc.vector.tensor_copy(out=kkT[:, tsl], in_=pk)
                # partition-shift qT up for the odd (row-packed) score matmuls
                nc.scalar.dma_start(out=qhT[H:P, tsl], in_=qvT[0:H, tsl])
                pvn = psc.tile([P, BPC, H], BF16, tag="sc")
                for tt in range(BPC):
                    c0 = tb * CH + tt * P
                    nc.tensor.transpose(pvn[:, tt, :], qvT[H:P, c0 : c0 + P],
                                        ident_b[H:P, H:P])
                nc.vector.tensor_copy(
                    out=v_all[:, tb * BPC : (tb + 1) * BPC, 0:H], in_=pvn)

            yield tail

        def attention(tb, pending):
            """Attention for chunk tb, weaving pending proj steps of tb+1 into
            the PE stream. exp runs on si pairs; even/odd score matmuls are
            row-packed into concurrent PE halves."""
            tsl = slice(tb * CH, (tb + 1) * CH)
            po = pout.tile([H + 1, CH], F32, tag="po")
            nsb = (tb + 1) * BPC
            wpairs = {}

            def weave():
                try:
                    next(pending)()
                except StopIteration:
                    pass

            def lo_of(si):
                return max(0, (si - tb * BPC) * P)

            for pr in range(nsb // 2 + 1):
                if pr < nsb // 2:
                    ps = psc.tile([P, 2, CH], F32, tag="sc")
                    w = wei.tile([P, 2, CH], BF16, tag="w")
                    se, so = 2 * pr, 2 * pr + 1
                    loe, loo = lo_of(se), lo_of(so)
                    # even s-block: PE rows 0-63; odd: rows 64-127 (concurrent)
                    nc.tensor.matmul(ps[:, 0, loe:CH],
                                     lhsT=kkT[0:H, se * P : (se + 1) * P],
                                     rhs=qT[:, tb * CH + loe : (tb + 1) * CH],
                                     start=True, stop=True)
                    nc.tensor.matmul(ps[:, 1, loo:CH],
                                     lhsT=kkT[H:P, so * P : (so + 1) * P],
                                     rhs=qhT[H:P, tb * CH + loo : (tb + 1) * CH],
                                     start=True, stop=True)
                    weave()
                    nc.scalar.activation(out=w, in_=ps, func=EXP, scale=SCALE)
                    for half in range(2):
                        si = 2 * pr + half
                        if si >= tb * BPC:  # diagonal 128x128: staircase mask
                            lo = lo_of(si)
                            nc.vector.tensor_mul(w[:, half, lo : lo + P],
                                                 w[:, half, lo : lo + P], tri)
                    wpairs[pr] = w
                if pr > 0:
                    w = wpairs.pop(pr - 1)
                    for half in range(2):
                        si = 2 * (pr - 1) + half
                        lo = lo_of(si)
                        nc.tensor.matmul(po[:, lo:CH], lhsT=v_all[:, si, :],
                                         rhs=w[:, half, lo:CH],
                                         start=(si == 0), stop=(si == nsb - 1))
                    weave()
            # finalize chunk: copy outT+sums to SBUF and store; the per-row
            # divide + transpose happens host-side during unshard.
            os_ = fin.tile([H + 1, CH], F32, tag="ot")
            nc.vector.tensor_copy(out=os_, in_=po)
            nc.gpsimd.dma_start(out=out_d[:, tsl], in_=os_)

        # chunk 0 projections run un-weaved; chunk tb+1 projections weave into
        # chunk tb's attention stream.
        for step in proj_steps(0):
            step()
        for tb in range(NCH):
            pending = proj_steps(tb + 1) if tb + 1 < NCH else iter(())
            attention(tb, pending)
            for step in pending:  # any proj work not yet woven
                step()
    return nc


_NC_CACHE = []


def _get_nc():
    if not _NC_CACHE:
        nc = build_nc()
        nc.finalize()  # bacc compile: register allocation, DCE
        _NC_CACHE.append(nc)
    return _NC_CACHE[0]


def _pack_inputs(x, wq, wk, wv):
    import ml_dtypes

    bf16 = np.dtype(ml_dtypes.bfloat16)
    # stationary packing: [c%128, c//128, h]; pass1 = [Wq | Wv], pass2 = [Wk | Wk]
    wq_p = wq.reshape(NC, P, H).transpose(1, 0, 2)
    wv_p = wv.reshape(NC, P, H).transpose(1, 0, 2)
    wk_p = wk.reshape(NC, P, H).transpose(1, 0, 2)
    wqv = np.ascontiguousarray(np.concatenate([wq_p, wv_p], axis=2)).astype(bf16)
    wkk = np.ascontiguousarray(np.concatenate([wk_p, wk_p], axis=2)).astype(bf16)
    # xt[b, tb, c%128, c//128, t_in_chunk] = x[b, tb*CH + t, c]
    xb = x.astype(bf16)                      # [B, T, C]
    xt = xb.transpose(0, 2, 1)               # [B, C, T]
    xt = xt.reshape(B, NC, P, NCH, CH)       # [B, jc, p, tb, t]
    xt = np.ascontiguousarray(xt.transpose(0, 3, 2, 1, 4))  # [B, tb, p, jc, t]
    return xt, wqv, wkk


def kernel(**inputs):
    x = np.asarray(inputs["x"], dtype=np.float32)
    wq = np.asarray(inputs["Wq"], dtype=np.float32)
    wk = np.asarray(inputs["Wk"], dtype=np.float32)
    wv = np.asarray(inputs["Wv"], dtype=np.float32)
    from concourse.bass_utils import run_bass_kernel_spmd

    nc = _get_nc()
    xb, wqv, wkk = _pack_inputs(x, wq, wk, wv)
    in_maps = [{"xt": np.ascontiguousarray(xb[b]), "wqv": wqv, "wkk": wkk}
               for b in range(B)]
    res = run_bass_kernel_spmd(nc, in_maps, core_ids=list(range(B)))
    return postprocess([res.results[b]["outT"] for b in range(B)])


def postprocess(outTs):
    outs = []
    for oT in outTs:
        outs.append((oT[0:H, :] / oT[H : H + 1, :]).T.astype(np.float32))
    return np.stack(outs, axis=0)


if __name__ == "__main__":
    import os
    os.makedirs("/tmp/neffdir3", exist_ok=True)
    from concourse.bass_utils import compile_bass_kernel

    nc = _get_nc()
    print("build OK, instructions:",
          sum(len(bb.instructions) for bb in nc.m.functions[0].blocks))
    print("COMPILED:", compile_bass_kernel(nc, "/tmp/neffdir3"))


# revision 33
# speedup vs baseline: 1.3500x; 1.0536x over previous
"""Single-head causal attention on 8 TRN2 NeuronCores.

Problem: x[8,2048,1024] @ Wq/Wk/Wv[1024,64] -> causal softmax attention -> out[8,2048,64].
Sharding: data-parallel over batch B=8, one batch element per core; weights replicated.

Per-core design v8 (T=2048, C=1024, H=64):
 - x and the weights are cast to bf16 and pre-packed on the HOST (numpy), so no
   on-chip casts and no software-DGE descriptor storms.
 - HYBRID xT production: chunks 0/1 arrive early via ordinary fast DMA
   (natural layout, Act queue) and are PE-transposed during otherwise-idle PE
   time; chunks 2/3 use the DMA XBAR transpose engine (SP queue), whose
   one-time ~7us ucode warmup is triggered by a tiny dummy transpose issued
   first, and whose ~160GB/s serial delivery hides behind chunk-0/1 compute.
 - projections per chunk: pass1 stationary [Wq|Wv], pass2 [Wk|Wk]. pass2 costs
   the same as [Wk] (matmul cost = moving width) but lands kT at BOTH PSUM
   partition ranges, so the score matmuls can be ROW-PACKED: even s-blocks run
   in PE rows 0-63 (base-0 kT/qT) CONCURRENTLY with odd s-blocks in rows
   64-127 (base-64 kT, and a per-chunk SBUF->SBUF DMA partition-shifts qT up).
 - q and v leave pass1 PSUM in a single [128,CH] copy into qvT (q rows 0:64
   persist, v rows 64:128 feed tile_position=(64,0) transposes into natural
   [s,64] blocks, extended with a ones column for the softmax denominator).
 - scores are computed TRANSPOSED: weiT[s, t-chunk] = kT_si.T @ qT, diagonal
   blocks at partial width [lo:CH]; exp runs on PAIRS of s-blocks as single
   1024-wide activations; left-of-diagonal w columns may hold exp(stale-PSUM)
   garbage which PV never reads (rhs sliced [lo:CH]); the diagonal 128x128
   gets the 0/1 staircase multiply.
 - PV uses stationary [v | 1] so PSUM row 64 accumulates the softmax
   denominators; projections (and chunk-1 transposes) of chunk tb+1 are
   interleaved into the attention stream of chunk tb so the PE never idles
   while ScalarE exps.
 - warmup matmuls on memset tiles run during the first DMA to flip the PE HAM
   clock-gate to 2.4 GHz before real work lands.
"""

import numpy as np

import concourse.bass as bass
import concourse.mybir as mybir
import concourse.tile as tile
from concourse import bacc
from concourse.masks import make_identity, make_upper_triangular
from contextlib import ExitStack

P = 128
T = 2048
C = 1024
H = 64
B = 8
NC = C // P          # 8 c-tiles
NT = T // P          # 16 s/t 128-blocks
CH = 512             # t-chunk width
NCH = T // CH        # 4 chunks
BPC = CH // P        # 4 blocks per chunk
HC = CH // 2         # half chunk (pipeline-fill XBAR splits)
SCALE = float(C) ** -0.5
F32 = mybir.dt.float32
BF16 = mybir.dt.bfloat16
EXP = mybir.ActivationFunctionType.Exp


def build_nc():
    nc = bacc.Bacc(None, target_bir_lowering=False)
    # host-transposed x tiles: xt[tb, c%128, c//128, t_in_chunk]
    xt_d = nc.dram_tensor("xt", [NCH, P, NC, CH], BF16, kind="ExternalInput")
    # host-packed stationaries: wqv[c%128, c//128, 0:64]=Wq, [.., 64:128]=Wv
    wqv_d = nc.dram_tensor("wqv", [P, NC, P], BF16, kind="ExternalInput")
    wkk_d = nc.dram_tensor("wkk", [P, NC, P], BF16, kind="ExternalInput")
    out_d = nc.dram_tensor("outT", [H + 1, T], F32, kind="ExternalOutput")

    with tile.TileContext(nc) as tc, ExitStack() as ctx:
        consts = ctx.enter_context(tc.tile_pool(name="consts", bufs=1))
        persist = ctx.enter_context(tc.tile_pool(name="persist", bufs=1))
        wei = ctx.enter_context(tc.tile_pool(name="wei", bufs=6))
        fin = ctx.enter_context(tc.tile_pool(name="fin", bufs=2))
        # PSUM: 8 banks, per (pool, tag): ppj/p1 1 + ppj/p2 1 +
        # psc/sc 2x2 + pout/po 2 = 8. The v mini-transposes borrow sc slots.
        ppj = ctx.enter_context(tc.tile_pool(name="ppj", bufs=1, space="PSUM"))
        psc = ctx.enter_context(tc.tile_pool(name="psc", bufs=2, space="PSUM"))
        pout = ctx.enter_context(tc.tile_pool(name="pout", bufs=2, space="PSUM"))

        # ---- DMAs first (no deps, start immediately).
        # SP queue: dummy XBAR (eats the one-time ucode warmup), then chunks
        # 2/3 XBAR transposes. Act queue: weights, then chunk 0/1 naturals.
        # XBAR transposes serialize on a shared ucode resource with a ~7us
        # one-time warmup: a tiny dummy transpose goes first so the warmup
        # overlaps the framework preamble; then chunks in order, 0/1 split in
        # halves so their projections can start per-piece.
        xTs = []
        xpieces = []  # per chunk: list of (col_offset, width)
        for tb in range(NCH):
            xTs.append(persist.tile([P, NC, CH], BF16, tag=f"xT{tb}",
                                    name=f"xT{tb}"))
            xpieces.append([(0, HC), (HC, HC)] if tb == 0 else [(0, CH)])
        wqv_sb = consts.tile([P, NC, P], BF16)
        nc.scalar.dma_start(out=wqv_sb, in_=wqv_d[:, :, :])
        wkk_sb = consts.tile([P, NC, P], BF16)
        nc.scalar.dma_start(out=wkk_sb, in_=wkk_d[:, :, :])
        # xT chunks are host-transposed and land via plain full-speed DMAs
        # (8KB contiguous per partition), alternating the two HWDGE queues.
        nc.sync.dma_start(out=xTs[0][:, :, 0:HC], in_=xt_d[0, :, :, 0:HC])
        nc.scalar.dma_start(out=xTs[0][:, :, HC:CH], in_=xt_d[0, :, :, HC:CH])
        for tb in range(1, NCH):
            eng = nc.sync if tb % 2 == 0 else nc.scalar
            eng.dma_start(out=xTs[tb], in_=xt_d[tb, :, :, :])

        # ---- constants
        ident_f = consts.tile([P, P], F32)
        make_identity(nc, ident_f)
        tri_f = consts.tile([P, P], F32)  # tri[s, u] = 1 if u >= s else 0
        make_upper_triangular(nc, tri_f, val=1.0, diag=True)
        ident_b = consts.tile([P, P], BF16)
        nc.vector.tensor_copy(out=ident_b, in_=ident_f)
        tri = consts.tile([P, P], BF16)
        nc.vector.tensor_copy(out=tri, in_=tri_f)

        # qvT: rows 0:64 = qT (persist), rows 64:128 = v transposed staging
        qvT = persist.tile([P, T], BF16, tag="qvT")
        qT = qvT[0:H, :]
        # kkT: rows 0:64 = kT for even s-blocks, rows 64:128 = same kT (for the
        # row-packed odd s-block matmuls at tile_position (64,0))
        kkT = persist.tile([P, T], BF16, tag="kkT")
        # qhT rows 64:128 = qT partition-shifted up via SBUF->SBUF DMA
        qhT = persist.tile([P, T], BF16, tag="qhT")
        v_all = persist.tile([P, NT, H + 1], BF16, tag="v")
        nc.vector.memset(v_all[:, :, H : H + 1], 1.0)  # softmax-denominator column

        # ---- HAM warmup: dummy matmuls on memset tiles (ready instantly) keep
        # the PE busy until chunk 0 lands so real matmuls run at 2.4 GHz.
        warm_lhs = consts.tile([P, P], BF16)
        nc.vector.memset(warm_lhs, 0.0)
        warm_in = consts.tile([P, CH], BF16)
        nc.vector.memset(warm_in, 0.0)
        # preload the ScalarE exp table (~2.7us) off the critical path
        warm_act = consts.tile([P, 1], BF16)
        nc.scalar.activation(out=warm_act, in_=warm_in[:, 0:1], func=EXP,
                             scale=1.0)
        pwarm = psc.tile([P, 2, CH], F32, tag="sc")
        for i in range(16):
            nc.tensor.matmul(pwarm[:, 0, :], lhsT=warm_lhs, rhs=warm_in,
                             start=True, stop=True)

        def proj_steps(tb):
            """Generator of projection work items for chunk tb (PE + DVE).
            Piece-wise over the chunk's XBAR arrivals, as ONE accumulation
            group per bank: start only on the very first matmul (clears the
            bank's has_written bits); later pieces' first writes hit
            bit-unset elements and overwrite, then accumulate."""
            tsl = slice(tb * CH, (tb + 1) * CH)
            pq = ppj.tile([P, CH], F32, tag="p1")
            pk = ppj.tile([P, CH], F32, tag="p2")
            xt = xTs[tb]
            pieces = xpieces[tb]
            np_ = len(pieces)
            for pi, (off, wd) in enumerate(pieces):
                for jc in range(NC):
                    first = pi == 0 and jc == 0
                    last = pi == np_ - 1 and jc == NC - 1
                    yield lambda off=off, wd=wd, jc=jc, first=first, last=last: (
                        nc.tensor.matmul(pq[:, off : off + wd],
                                         lhsT=wqv_sb[:, jc, :],
                                         rhs=xt[:, jc, off : off + wd],
                                         start=first, stop=last,
                                         skip_group_check=True),
                        nc.tensor.matmul(pk[:, off : off + wd],
                                         lhsT=wkk_sb[:, jc, :],
                                         rhs=xt[:, jc, off : off + wd],
                                         start=first, stop=last,
                                         skip_group_check=True),
                    )

            def tail():
                # q (rows 0:64) and v (rows 64:128) leave PSUM in one copy;
                # kT lands at both partition ranges in one copy.
                nc.vector.tensor_copy(out=qvT[:, tsl], in_=pq)
                nc.vector.tensor_copy(out=kkT[:, tsl], in_=pk)
                # partition-shift qT up for the odd (row-packed) score matmuls
                nc.scalar.dma_start(out=qhT[H:P, tsl], in_=qvT[0:H, tsl])
                pvn = psc.tile([P, BPC, H], BF16, tag="sc")
                for tt in range(BPC):
                    c0 = tb * CH + tt * P
                    nc.tensor.transpose(pvn[:, tt, :], qvT[H:P, c0 : c0 + P],
                                        ident_b[H:P, H:P])
                nc.vector.tensor_copy(
                    out=v_all[:, tb * BPC : (tb + 1) * BPC, 0:H], in_=pvn)

            yield tail

        def attention(tb, pending):
            """Attention for chunk tb, weaving pending proj steps of tb+1 into
            the PE stream. exp runs on si pairs; even/odd score matmuls are
            row-packed into concurrent PE halves."""
            tsl = slice(tb * CH, (tb + 1) * CH)
            po = pout.tile([H + 1, CH], F32, tag="po")
            nsb = (tb + 1) * BPC
            wpairs = {}

            def weave():
                for _ in range(2):
                    try:
                        next(pending)()
                    except StopIteration:
                        return

            def lo_of(si):
                return max(0, (si - tb * BPC) * P)

            for pr in range(nsb // 2 + 1):
                if pr < nsb // 2:
                    ps = psc.tile([P, 2, CH], F32, tag="sc")
                    w = wei.tile([P, 2, CH], BF16, tag="w")
                    se, so = 2 * pr, 2 * pr + 1
                    loe, loo = lo_of(se), lo_of(so)
                    # even s-block: PE rows 0-63; odd: rows 64-127 (concurrent)
                    nc.tensor.matmul(ps[:, 0, loe:CH],
                                     lhsT=kkT[0:H, se * P : (se + 1) * P],
                                     rhs=qT[:, tb * CH + loe : (tb + 1) * CH],
                                     start=True, stop=True)
                    nc.tensor.matmul(ps[:, 1, loo:CH],
                                     lhsT=kkT[H:P, so * P : (so + 1) * P],
                                     rhs=qhT[H:P, tb * CH + loo : (tb + 1) * CH],
                                     start=True, stop=True)
                    weave()
                    nc.scalar.activation(out=w, in_=ps, func=EXP, scale=SCALE)
                    for half in range(2):
                        si = 2 * pr + half
                        if si >= tb * BPC:  # diagonal 128x128: staircase mask
                            lo = lo_of(si)
                            nc.vector.tensor_mul(w[:, half, lo : lo + P],
                                                 w[:, half, lo : lo + P], tri)
                    wpairs[pr] = w
                if pr > 0:
                    w = wpairs.pop(pr - 1)
                    for half in range(2):
                        si = 2 * (pr - 1) + half
                        lo = lo_of(si)
                        nc.tensor.matmul(po[:, lo:CH], lhsT=v_all[:, si, :],
                                         rhs=w[:, half, lo:CH],
                                         start=(si == 0), stop=(si == nsb - 1))
                    weave()
            # finalize chunk: copy outT+sums to SBUF and store; the per-row
            # divide + transpose happens host-side during unshard.
            os_ = fin.tile([H + 1, CH], F32, tag="ot")
            nc.vector.tensor_copy(out=os_, in_=po)
            nc.sync.dma_start(out=out_d[:, tsl], in_=os_)

        # chunk 0 projections run un-weaved; chunk tb+1 projections weave into
        # chunk tb's attention stream.
        for step in proj_steps(0):
            step()
        for tb in range(NCH):
            pending = proj_steps(tb + 1) if tb + 1 < NCH else iter(())
            attention(tb, pending)
            for step in pending:  # any proj work not yet woven
                step()
    return nc


_NC_CACHE = []


def _get_nc():
    if not _NC_CACHE:
        nc = build_nc()
        nc.finalize()  # bacc compile: register allocation, DCE
        _NC_CACHE.append(nc)
    return _NC_CACHE[0]


def _pack_inputs(x, wq, wk, wv):
    import ml_dtypes

    bf16 = np.dtype(ml_dtypes.bfloat16)
    # stationary packing: [c%128, c//128, h]; pass1 = [Wq | Wv], pass2 = [Wk | Wk]
    wq_p = wq.reshape(NC, P, H).transpose(1, 0, 2)
    wv_p = wv.reshape(NC, P, H).transpose(1, 0, 2)
    wk_p = wk.reshape(NC, P, H).transpose(1, 0, 2)
    wqv = np.ascontiguousarray(np.concatenate([wq_p, wv_p], axis=2)).astype(bf16)
    wkk = np.ascontiguousarray(np.concatenate([wk_p, wk_p], axis=2)).astype(bf16)
    # xt[b, tb, c%128, c//128, t_in_chunk] = x[b, tb*CH + t, c]
    xb = x.astype(bf16)                      # [B, T, C]
    xt = xb.transpose(0, 2, 1)               # [B, C, T]
    xt = xt.reshape(B, NC, P, NCH, CH)       # [B, jc, p, tb, t]
    xt = np.ascontiguousarray(xt.transpose(0, 3, 2, 1, 4))  # [B, tb, p, jc, t]
    return xt, wqv, wkk


def kernel(**inputs):
    x = np.asarray(inputs["x"], dtype=np.float32)
    wq = np.asarray(inputs["Wq"], dtype=np.float32)
    wk = np.asarray(inputs["Wk"], dtype=np.float32)
    wv = np.asarray(inputs["Wv"], dtype=np.float32)
    from concourse.bass_utils import run_bass_kernel_spmd

    nc = _get_nc()
    xb, wqv, wkk = _pack_inputs(x, wq, wk, wv)
    in_maps = [{"xt": np.ascontiguousarray(xb[b]), "wqv": wqv, "wkk": wkk}
               for b in range(B)]
    res = run_bass_kernel_spmd(nc, in_maps, core_ids=list(range(B)))
    return postprocess([res.results[b]["outT"] for b in range(B)])


def postprocess(outTs):
    outs = []
    for oT in outTs:
        outs.append((oT[0:H, :] / oT[H : H + 1, :]).T.astype(np.float32))
    return np.stack(outs, axis=0)


if __name__ == "__main__":
    import os
    os.makedirs("/tmp/neffdir3", exist_ok=True)
    from concourse.bass_utils import compile_bass_kernel

    nc = _get_nc()
    print("build OK, instructions:",
          sum(len(bb.instructions) for bb in nc.m.functions[0].blocks))
    print("COMPILED:", compile_bass_kernel(nc, "/tmp/neffdir3"))
